# revision 3
# baseline (speedup 1.0000x reference)
"""Multi-head attention Bass kernel for TRN2, 8-core SPMD.

Sharding: core i handles batch b = i//2, query rows [1024*(i%2), +1024).
Each core computes K/V for all 2048 keys of its batch and its 1024 query rows
end-to-end. No collectives; the host concatenates the 8 [1024,1024] outputs.

Precision: Q/K projections and QK^T in bf16x3 (hi+lo split, 3 matmuls);
V path in bf16. Softmax: shift by an approximate row max from a 1-pass bf16
score preview (softmax is shift-invariant; |s_bf16 - s| <= ~32 << 80 so
exp stays in fp32 range), exp on ACT with accumulated sums, normalize P.

Head pairs are processed in 2 groups of 4 so qT/kT/v SBUF residency fits.
"""
import sys
sys.path.insert(0, "/opt/trn_rl_repo")
import numpy as np
import ml_dtypes
from concourse import bass, bacc, tile, mybir
from concourse.bass_utils import run_bass_kernel_spmd

BF16 = mybir.dt.bfloat16
F32 = mybir.dt.float32
AF = mybir.ActivationFunctionType
ALU = mybir.AluOpType

B, S, D = 4, 2048, 1024
H, DK, DV = 16, 64, 64
NCORES = 8
SQ = 1024


def np_bf16_split(a):
    hi = a.astype(ml_dtypes.bfloat16)
    lo = (a.astype(np.float32) - hi.astype(np.float32)).astype(ml_dtypes.bfloat16)
    return hi, lo


def host_prep(x, W_q, W_k, W_v, W_o):
    x = np.asarray(x, np.float32)

    def pack_w(W):  # [H, D, 64] -> [128 dlo, 8 dhi, 8 pair, 128]
        Wp = np.stack([np.concatenate([W[2 * p], W[2 * p + 1]], axis=1)
                       for p in range(8)], axis=1)     # [D, 8, 128]
        return np.ascontiguousarray(
            Wp.reshape(8, 128, 8, 128).transpose(1, 0, 2, 3))

    wq_hi, wq_lo = np_bf16_split(pack_w(np.asarray(W_q, np.float32)))
    wk_hi, wk_lo = np_bf16_split(pack_w(np.asarray(W_k, np.float32)))
    wv_hi, _ = np_bf16_split(pack_w(np.asarray(W_v, np.float32)))
    wo = np.asarray(W_o, np.float32).reshape(8, 128, 1024).transpose(1, 0, 2)
    wo_hi = np.ascontiguousarray(wo.astype(ml_dtypes.bfloat16))

    in_maps = []
    for core in range(NCORES):
        b, half = core // 2, core % 2
        xk = np.ascontiguousarray(
            x[b].T.reshape(8, 128, S).transpose(1, 0, 2))   # [128, 8, 2048]
        xq = np.ascontiguousarray(xk[:, :, half * SQ:(half + 1) * SQ])
        xk_hi, xk_lo = np_bf16_split(xk)
        xq_hi, xq_lo = np_bf16_split(xq)
        in_maps.append({
            "xk_hi": xk_hi, "xk_lo": xk_lo, "xq_hi": xq_hi, "xq_lo": xq_lo,
            "wq_hi": wq_hi, "wq_lo": wq_lo, "wk_hi": wk_hi, "wk_lo": wk_lo,
            "wv_hi": wv_hi, "wo_hi": wo_hi,
        })
    return in_maps


def host_gather(results):
    out = np.empty((B, S, D), np.float32)
    for core, r in enumerate(results):
        b, half = core // 2, core % 2
        out[b, half * SQ:(half + 1) * SQ, :] = r["out"]
    return out


def build_program(n_groups=2, pairs_per_group=4, n_sqtiles=8, n_tchunks=4,
                  debug=False):
    NT = n_tchunks * 512
    NSQ = n_sqtiles * 128
    n_ttiles = n_tchunks * 4
    n_sq512 = n_sqtiles // 4
    NPAIR = n_groups * pairs_per_group
    NH = NPAIR * 2
    assert n_sqtiles % 4 == 0

    nc = bacc.Bacc("TRN2", target_bir_lowering=False, debug=False,
                   num_devices=NCORES)

    dram = {}
    for nm in ["xk_hi", "xk_lo"]:
        dram[nm] = nc.dram_tensor(nm, [128, 8, S], BF16, kind="ExternalInput")
    for nm in ["xq_hi", "xq_lo"]:
        dram[nm] = nc.dram_tensor(nm, [128, 8, SQ], BF16, kind="ExternalInput")
    for nm in ["wq_hi", "wq_lo", "wk_hi", "wk_lo", "wv_hi"]:
        dram[nm] = nc.dram_tensor(nm, [128, 8, 8, 128], BF16, kind="ExternalInput")
    dram["wo_hi"] = nc.dram_tensor("wo_hi", [128, 8, 1024], BF16,
                                   kind="ExternalInput")
    out_ext = nc.dram_tensor("out", [SQ, D], F32, kind="ExternalOutput")
    dbg = {}
    if debug:
        dbg["qT_hi"] = nc.dram_tensor("dbg_qT_hi", [128, pairs_per_group, NSQ],
                                      BF16, kind="ExternalOutput")
        dbg["kT_hi"] = nc.dram_tensor("dbg_kT_hi", [128, pairs_per_group, NT],
                                      BF16, kind="ExternalOutput")
        dbg["v"] = nc.dram_tensor("dbg_v", [128, n_ttiles, pairs_per_group * 128],
                                  BF16, kind="ExternalOutput")
        dbg["sig"] = nc.dram_tensor("dbg_sig", [128, 128], F32,
                                    kind="ExternalOutput")
        dbg["bias"] = nc.dram_tensor("dbg_bias", [128, 128], F32,
                                     kind="ExternalOutput")
        dbg["P"] = nc.dram_tensor("dbg_P", [128, NT], BF16, kind="ExternalOutput")
        dbg["cT"] = nc.dram_tensor("dbg_cT", [128, NPAIR, NSQ], BF16,
                                   kind="ExternalOutput")

    with tile.TileContext(nc) as tc:
        with tc.tile_pool(name="persist", bufs=1) as pp, \
             tc.tile_pool(name="stream", bufs=1) as sp, \
             tc.tile_pool(name="wstream", bufs=2) as wsp, \
             tc.tile_pool(name="work", bufs=2) as wp, \
             tc.tile_pool(name="ptp", bufs=1) as ptp, \
             tc.tile_pool(name="ps", bufs=4, space="PSUM") as ps, \
             tc.tile_pool(name="pso", bufs=2, space="PSUM") as pso:

            concT = pp.tile([128, NPAIR, NSQ], BF16, tag="concT")
            sig_all = pp.tile([128, 128], F32, tag="sig_all")
            bias_all = pp.tile([128, 128], F32, tag="bias_all")

            for g in range(n_groups):
                # group-scoped tensors (tags reused across groups)
                qT_hi = pp.tile([128, pairs_per_group, NSQ], BF16, tag="qT_hi")
                qT_lo = pp.tile([128, pairs_per_group, NSQ], BF16, tag="qT_lo")
                kT_hi = pp.tile([128, pairs_per_group, NT], BF16, tag="kT_hi")
                kT_lo = pp.tile([128, pairs_per_group, NT], BF16, tag="kT_lo")
                v_sb = pp.tile([128, n_ttiles, pairs_per_group * 128], BF16,
                               tag="v_sb")

                # ---------- projections for this group's pairs ----------
                for tc_i in range(n_tchunks):
                    xs_hi = sp.tile([128, 8, 512], BF16, tag="xs_hi")
                    xs_lo = sp.tile([128, 8, 512], BF16, tag="xs_lo")
                    nc.sync.dma_start(xs_hi[:], dram["xk_hi"][:, :, bass.ts(tc_i, 512)])
                    nc.sync.dma_start(xs_lo[:], dram["xk_lo"][:, :, bass.ts(tc_i, 512)])
                    for lp in range(pairs_per_group):
                        p = g * pairs_per_group + lp
                        wkh = wsp.tile([128, 8, 128], BF16, tag="wkh")
                        wkl = wsp.tile([128, 8, 128], BF16, tag="wkl")
                        wvh = wsp.tile([128, 8, 128], BF16, tag="wvh")
                        nc.sync.dma_start(wkh[:], dram["wk_hi"][:, :, p, :])
                        nc.sync.dma_start(wkl[:], dram["wk_lo"][:, :, p, :])
                        nc.sync.dma_start(wvh[:], dram["wv_hi"][:, :, p, :])
                        kp = ps.tile([128, 512], F32, tag="mm512")
                        for d in range(8):
                            nc.tensor.matmul(kp[:], wkh[:, d, :], xs_hi[:, d, :],
                                             start=(d == 0), stop=False)
                            nc.tensor.matmul(kp[:], wkh[:, d, :], xs_lo[:, d, :],
                                             start=False, stop=False)
                            nc.tensor.matmul(kp[:], wkl[:, d, :], xs_hi[:, d, :],
                                             start=False, stop=(d == 7))
                        dst_hi = kT_hi[:, lp, bass.ts(tc_i, 512)]
                        dst_lo = kT_lo[:, lp, bass.ts(tc_i, 512)]
                        tmp = wp.tile([128, 512], F32, tag="split_tmp")
                        nc.vector.tensor_copy(dst_hi, kp[:])
                        nc.vector.tensor_sub(tmp[:], kp[:], dst_hi)
                        nc.vector.tensor_copy(dst_lo, tmp[:])
                        for tt in range(4):
                            vp = ps.tile([128, 512], F32, tag="mm512")
                            for d in range(8):
                                nc.tensor.matmul(
                                    vp[:, :128], xs_hi[:, d, bass.ts(tt, 128)],
                                    wvh[:, d, :], start=(d == 0), stop=(d == 7))
                            nc.vector.tensor_copy(
                                v_sb[:, tc_i * 4 + tt, bass.ts(lp, 128)],
                                vp[:, :128])

                for qc in range(n_sqtiles // 4):
                    xs_hi = sp.tile([128, 8, 512], BF16, tag="xs_hi")
                    xs_lo = sp.tile([128, 8, 512], BF16, tag="xs_lo")
                    nc.sync.dma_start(xs_hi[:], dram["xq_hi"][:, :, bass.ts(qc, 512)])
                    nc.sync.dma_start(xs_lo[:], dram["xq_lo"][:, :, bass.ts(qc, 512)])
                    for lp in range(pairs_per_group):
                        p = g * pairs_per_group + lp
                        wqh = wsp.tile([128, 8, 128], BF16, tag="wkh")
                        wql = wsp.tile([128, 8, 128], BF16, tag="wkl")
                        nc.sync.dma_start(wqh[:], dram["wq_hi"][:, :, p, :])
                        nc.sync.dma_start(wql[:], dram["wq_lo"][:, :, p, :])
                        qp = ps.tile([128, 512], F32, tag="mm512")
                        for d in range(8):
                            nc.tensor.matmul(qp[:], wqh[:, d, :], xs_hi[:, d, :],
                                             start=(d == 0), stop=False)
                            nc.tensor.matmul(qp[:], wqh[:, d, :], xs_lo[:, d, :],
                                             start=False, stop=False)
                            nc.tensor.matmul(qp[:], wql[:, d, :], xs_hi[:, d, :],
                                             start=False, stop=(d == 7))
                        dst_hi = qT_hi[:, lp, bass.ts(qc, 512)]
                        dst_lo = qT_lo[:, lp, bass.ts(qc, 512)]
                        tmp = wp.tile([128, 512], F32, tag="split_tmp")
                        nc.vector.tensor_copy(dst_hi, qp[:])
                        nc.vector.tensor_sub(tmp[:], qp[:], dst_hi)
                        nc.vector.tensor_copy(dst_lo, tmp[:])

                if debug and g == 0:
                    nc.sync.dma_start(dbg["qT_hi"][:], qT_hi[:])
                    nc.sync.dma_start(dbg["kT_hi"][:], kT_hi[:])
                    nc.sync.dma_start(dbg["v"][:], v_sb[:])

                # ---------- attention for this group ----------
                for lp in range(pairs_per_group):
                    p = g * pairs_per_group + lp
                    for sqc in range(n_sq512):
                        PT = [ptp.tile([128, n_ttiles, 512], BF16,
                                       tag=f"PT{hh}", name=f"PT{hh}")
                              for hh in range(2)]
                        for s4 in range(4):
                            sq = sqc * 4 + s4
                            h0, h1 = 2 * p, 2 * p + 1
                            c0, c1 = sq * 16 + h0, sq * 16 + h1
                            qs = bass.ts(sq, 128)
                            q_hi = [qT_hi[0:64, lp, qs], qT_hi[64:128, lp, qs]]
                            q_lo = [qT_lo[0:64, lp, qs], qT_lo[64:128, lp, qs]]
                            # approx max: 1-pass bf16 scores, heads interleaved
                            mxp = [wp.tile([128, n_tchunks], F32, tag="mxp0",
                                           name="mxp0"),
                                   wp.tile([128, n_tchunks], F32, tag="mxp1",
                                           name="mxp1")]
                            for tc_i in range(n_tchunks):
                                a_ps = [ps.tile([128, 512], F32, tag="mm512",
                                                name="a_ps0"),
                                        ps.tile([128, 512], F32, tag="mm512",
                                                name="a_ps1")]
                                for hh in range(2):
                                    b0, b1 = (0, 64) if hh == 0 else (64, 128)
                                    nc.tensor.matmul(
                                        a_ps[hh][:], q_hi[hh],
                                        kT_hi[b0:b1, lp, bass.ts(tc_i, 512)],
                                        start=True, stop=True)
                                for hh in range(2):
                                    nc.vector.tensor_reduce(
                                        mxp[hh][:, tc_i:tc_i + 1], a_ps[hh][:],
                                        mybir.AxisListType.X, ALU.max)
                            for hh, col in ((0, c0), (1, c1)):
                                mx = wp.tile([128, 1], F32, tag="mx")
                                nc.vector.tensor_reduce(
                                    mx[:], mxp[hh][:], mybir.AxisListType.X,
                                    ALU.max)
                                nc.vector.tensor_scalar_mul(
                                    bias_all[:, col:col + 1], mx[:], -0.125)
                            # exact scores (bf16x3), heads interleaved per chunk
                            P_sb = [wp.tile([128, NT], BF16, tag="P0", name="P0"),
                                    wp.tile([128, NT], BF16, tag="P1", name="P1")]
                            sgp = [wp.tile([128, n_tchunks], F32, tag="sgp0",
                                           name="sgp0"),
                                   wp.tile([128, n_tchunks], F32, tag="sgp1",
                                           name="sgp1")]
                            for tc_i in range(n_tchunks):
                                s_ps = [ps.tile([128, 512], F32, tag="mm512",
                                                name="s_ps0"),
                                        ps.tile([128, 512], F32, tag="mm512",
                                                name="s_ps1")]
                                kk = bass.ts(tc_i, 512)
                                for hh in range(2):
                                    b0, b1 = (0, 64) if hh == 0 else (64, 128)
                                    nc.tensor.matmul(s_ps[hh][:], q_hi[hh],
                                                     kT_hi[b0:b1, lp, kk],
                                                     start=True, stop=False)
                                for hh in range(2):
                                    b0, b1 = (0, 64) if hh == 0 else (64, 128)
                                    nc.tensor.matmul(s_ps[hh][:], q_hi[hh],
                                                     kT_lo[b0:b1, lp, kk],
                                                     start=False, stop=False)
                                for hh in range(2):
                                    b0, b1 = (0, 64) if hh == 0 else (64, 128)
                                    nc.tensor.matmul(s_ps[hh][:], q_lo[hh],
                                                     kT_hi[b0:b1, lp, kk],
                                                     start=False, stop=True)
                                for hh, col in ((0, c0), (1, c1)):
                                    nc.scalar.activation(
                                        P_sb[hh][:, kk], s_ps[hh][:], AF.Exp,
                                        bias=bias_all[:, col:col + 1],
                                        scale=0.125,
                                        accum_out=sgp[hh][:, tc_i:tc_i + 1])
                            for hh, col in ((0, c0), (1, c1)):
                                nc.vector.tensor_reduce(
                                    sig_all[:, col:col + 1], sgp[hh][:],
                                    mybir.AxisListType.X, ALU.add)
                                rec = wp.tile([128, 1], F32, tag="rec")
                                nc.vector.reciprocal(rec[:],
                                                     sig_all[:, col:col + 1])
                                nc.vector.tensor_scalar_mul(P_sb[hh][:],
                                                            P_sb[hh][:], rec[:])
                                if debug and p == 0 and sq == 0 and hh == 0:
                                    nc.sync.dma_start(dbg["P"][:], P_sb[0][:])
                                eng = nc.scalar if hh == 0 else nc.sync
                                eng.dma_start_transpose(
                                    PT[hh][:, :, bass.ts(s4, 128)], P_sb[hh][:])
                        # attn @ V: both heads col-packed into one psum
                        o_ps = pso.tile([128, 512], F32, tag="mmo", name="o_ps")
                        for j in range(n_ttiles):
                            for hh in range(2):
                                b0, b1 = (0, 64) if hh == 0 else (64, 128)
                                nc.tensor.matmul(
                                    o_ps[b0:b1, :],
                                    v_sb[:, j, lp * 128 + hh * 64:
                                         lp * 128 + hh * 64 + 64],
                                    PT[hh][:, j, :],
                                    start=(j == 0), stop=(j == n_ttiles - 1))
                        nc.vector.tensor_copy(
                            concT[:, p, bass.ts(sqc, 512)], o_ps[:])

            if debug:
                nc.sync.dma_start(dbg["sig"][:], sig_all[:])
                nc.sync.dma_start(dbg["bias"][:], bias_all[:])
                nc.sync.dma_start(dbg["cT"][:], concT[:])

            # ---------- output projection ----------
            wo_sb = sp.tile([128, 8, 1024], BF16, tag="xs_hi")
            nc.gpsimd.dma_start(wo_sb[:], dram["wo_hi"][:])
            for sq in range(n_sqtiles):
                outp = [ps.tile([128, 512], F32, tag="mm512", name=f"outp{_i}")
                        for _i in range(2)]
                for p in range(NPAIR):
                    lhs = concT[:, p, bass.ts(sq, 128)]
                    for dc in range(2):
                        nc.tensor.matmul(outp[dc][:], lhs,
                                         wo_sb[:, p, bass.ts(dc, 512)],
                                         start=(p == 0), stop=(p == NPAIR - 1))
                o_sb = wp.tile([128, 1024], F32, tag="o_sb")
                for dc in range(2):
                    nc.vector.tensor_copy(o_sb[:, bass.ts(dc, 512)], outp[dc][:])
                nc.gpsimd.dma_start(out_ext[bass.ts(sq, 128), :], o_sb[:])

    nc.finalize()
    return nc


def run(nc, in_maps, trace=False):
    return run_bass_kernel_spmd(nc, in_maps, list(range(NCORES)), trace=trace)


# ----------------------------------------------------------------------------
# Harness entry point: kernel(**inputs) -> full [B, S, D] output.
# ----------------------------------------------------------------------------
import os

_nc_cache = {}
_last_exec_ns = [None]


def last_exec_time_ns():
    return _last_exec_ns[0]


def kernel(x, W_q, W_k, W_v, W_o):
    trace = bool(int(os.environ.get("KERNEL_TRACE", "0")))
    if "nc" not in _nc_cache:
        _nc_cache["nc"] = build_program()
    nc = _nc_cache["nc"]
    in_maps = host_prep(x, W_q, W_k, W_v, W_o)
    res = run_bass_kernel_spmd(nc, in_maps, list(range(NCORES)), trace=trace)
    _last_exec_ns[0] = res.exec_time_ns
    return host_gather(res.results)
